# revision 1
# baseline (speedup 1.0000x reference)
"""Trainium2 Bass kernel for BertLinearSelfAttention (linear attention).

Reference computation (per batch b, head h):
    q,k,v = X @ W{q,k,v} + b{q,k,v}            # [S, D] -> heads of 64
    qf, kf = elu(q)+1, elu(k)+1                # = min(exp(x),1) + max(x,0)
    kv[d,e]  = sum_s kf[s,d] v[s,e]            # [64, 64]
    ksum[d]  = sum_s kf[s,d]
    out[s,e] = (sum_d qf[s,d] kv[d,e]) / (sum_d qf[s,d] ksum[d])

Sharding: 8 cores = (4 batches) x (2 head-groups of 8 heads / 512 proj cols).

All matmuls in bf16 (1 col/cycle, same PE rate as fp32r; the 2e-2 rel-err
gate leaves ~50x headroom over bf16's ~0.5% noise). Single pass over X:
per 512-token chunk compute k/v/q projections + feature maps, accumulate
kv/ksum, and stash q-features (bf16, 4.2MB) in SBUF. The v bias is folded
into kv afterwards as the rank-1 update ksum x bv, so the v path needs only
a PSUM->SBUF copy (on ACT). Tail: per chunk, block-diagonal num/den matmuls
+ reciprocal + broadcast multiply (split DVE/ACT/GPS) + bf16 store.
"""

import os
import sys

import numpy as np
import ml_dtypes

_REPO = "/opt/trn_rl_repo"
if os.path.isdir(_REPO) and _REPO not in sys.path:
    sys.path.insert(0, _REPO)

B, S, D, H, HD = 4, 4096, 1024, 16, 64
NCORES = 8
CG = 512            # projection columns per core (8 heads)
NH = CG // HD       # 8 heads per core
HE = HD + 2         # head cols incl ksum column + pad
CHUNK = 512         # tokens per chunk
NSUB = CHUNK // 128     # 4 token sub-tiles per chunk
NCHUNK = S // CHUNK     # 8 chunks
NKT = D // 128          # 8 contraction tiles
P = 128
NCT = CG // P           # 4 column tiles (2 heads each)

BF16 = ml_dtypes.bfloat16

_CACHED_NC = None


def _build():
    import concourse.tile as tile
    from concourse import bacc, mybir
    from contextlib import ExitStack

    F32 = mybir.dt.float32
    BF = mybir.dt.bfloat16
    Alu = mybir.AluOpType
    Act = mybir.ActivationFunctionType

    nc = bacc.Bacc("TRN2", target_bir_lowering=False, debug=False,
                   num_devices=NCORES)

    # host-packed layouts: rows are SBUF partitions, cols kt-major — every
    # load is one 2D DMA with 128 contiguous multi-KB runs
    xt_d = nc.dram_tensor("xt", [NCHUNK * P, NKT * CHUNK], BF,
                          kind="ExternalInput").ap()
    wk_d = nc.dram_tensor("wk", [P, NKT * CG], BF, kind="ExternalInput").ap()
    wv_d = nc.dram_tensor("wv", [P, NKT * CG], BF, kind="ExternalInput").ap()
    wq_d = nc.dram_tensor("wq", [P, NKT * CG], BF, kind="ExternalInput").ap()
    bq_d = nc.dram_tensor("bq", [CG], F32, kind="ExternalInput").ap()
    bk_d = nc.dram_tensor("bk", [1, CG], BF, kind="ExternalInput").ap()
    bv_d = nc.dram_tensor("bv", [1, NH * HD], F32, kind="ExternalInput").ap()
    out_d = nc.dram_tensor("out", [S, CG], BF, kind="ExternalOutput").ap()

    with tile.TileContext(nc) as tc:
        with ExitStack() as ctx:
            const = ctx.enter_context(tc.tile_pool(name="const", bufs=1))
            wpool = ctx.enter_context(tc.tile_pool(name="wpool", bufs=1))
            xtpool = ctx.enter_context(tc.tile_pool(name="xtpool", bufs=3))

            w_sb = {}
            for nm in ("k", "v", "q"):
                w_sb[nm] = wpool.tile([P, NKT * CG], BF, tag=f"w{nm}",
                                      name=f"w{nm}")

            def load_xt(ci, eng=None):
                # split so neither piece is a fully-contiguous DRAM region:
                # whole-slice sources get merged to 1D by the DMA lowering
                # and re-split in an order that scrambles the SBUF dest
                t = xtpool.tile([P, NKT * CHUNK], BF, tag="xt", name="xt")
                eng = eng or nc.sync
                eng.dma_start(t[:, :CHUNK], xt_d[ci * P:(ci + 1) * P, :CHUNK])
                eng.dma_start(t[:, CHUNK:], xt_d[ci * P:(ci + 1) * P, CHUNK:])
                return t

            # startup: smallest pieces first so the first matmul starts ASAP,
            # weights and xt split across queues to parallelize issue
            nc.sync.dma_start(w_sb["k"][:, :CG], wk_d[:, :CG])
            xtb0 = load_xt(0, nc.sync)
            nc.sync.dma_start(w_sb["k"][:, CG:], wk_d[:, CG:])
            xtb1 = load_xt(1, nc.gpsimd)
            nc.gpsimd.dma_start(w_sb["v"][:, :CG], wv_d[:, :CG])
            nc.gpsimd.dma_start(w_sb["v"][:, CG:], wv_d[:, CG:])
            nc.gpsimd.dma_start(w_sb["q"][:, :CG], wq_d[:, :CG])
            nc.gpsimd.dma_start(w_sb["q"][:, CG:], wq_d[:, CG:])

            # ---- small constants ----
            bk_r = const.tile([1, CG], BF, tag="bkr")
            nc.sync.dma_start(bk_r[:], bk_d[:])
            bk_rep = const.tile([P, CG], BF, tag="bkrep")
            nc.gpsimd.partition_broadcast(bk_rep[:], bk_r[:])
            bq_sb = const.tile([P, NCT], F32, tag="bqsb")
            nc.sync.dma_start(bq_sb[:], bq_d.rearrange("(c p) -> p c", p=P))
            bv_sb = const.tile([1, NH * HD], F32, tag="bv32")
            nc.sync.dma_start(bv_sb[:], bv_d[:])
            bv_rep = const.tile([P, NH * HD], F32, tag="bvrep")
            nc.gpsimd.partition_broadcast(bv_rep[:], bv_sb[:])

            # kv accumulator (SBUF f32): per head [64, HE] (ksum in col HD)
            kv_sb = wpool.tile([HD, NH * HE], F32, tag="kvsb")
            nc.vector.memset(kv_sb[:], 0.0)

            # block-diagonal num weights [128,128] per ct + den cols [128,2]
            kvbn = [wpool.tile([P, P], BF, tag=f"kvbn{i}", name=f"kvbn{i}")
                    for i in range(NCT)]
            kvbd = [wpool.tile([P, 2], BF, tag=f"kvbd{i}", name=f"kvbd{i}")
                    for i in range(NCT)]
            for t in kvbn + kvbd:
                nc.vector.memset(t[:], 0.0)

            # persistent V' tiles (2 chunks' worth): tail cols preset once
            vp_tiles = [wpool.tile([P, NH * HE], BF, tag=f"vp{i}",
                                   name=f"vp{i}") for i in range(2 * NSUB)]
            for t in vp_tiles:
                nc.vector.memset(
                    t[:].rearrange("p (h e) -> p h e", e=HE)[:, :, HD:], 0.0)
                nc.vector.memset(
                    t[:].rearrange("p (h e) -> p h e", e=HE)[:, :, HD:HD + 1],
                    1.0)

            # q-feature store for the whole sequence (bf16, 4.2MB)
            qft_all = wpool.tile([P, NCHUNK * NCT * CHUNK], BF, tag="qft")

            kfpool = ctx.enter_context(tc.tile_pool(name="kfpool", bufs=10))
            tmp = ctx.enter_context(tc.tile_pool(name="tmp", bufs=12))
            outpool = ctx.enter_context(tc.tile_pool(name="outp", bufs=6))
            rcpool = ctx.enter_context(tc.tile_pool(name="rcp", bufs=8))
            pps = ctx.enter_context(
                tc.tile_pool(name="pps", bufs=6, space="PSUM"))
            sps = ctx.enter_context(
                tc.tile_pool(name="sps", bufs=2, space="PSUM"))

            def build_kvblocks():
                # block-diagonal kv (with rank-1 bv fix) + den columns
                for ct in range(NCT):
                    for half in range(2):
                        h = 2 * ct + half
                        dst = kvbn[ct][half * HD:(half + 1) * HD,
                                       half * HD:(half + 1) * HD]
                        ks_col = kv_sb[:, h * HE + HD:h * HE + HD + 1]
                        # kv_fixed = bv_h * ksum_h + kv_h  (rank-1 bias fold)
                        nc.vector.scalar_tensor_tensor(
                            dst, bv_rep[0:HD, h * HD:(h + 1) * HD], ks_col,
                            kv_sb[:, h * HE:h * HE + HD], Alu.mult, Alu.add)
                        nc.vector.tensor_copy(
                            kvbd[ct][half * HD:(half + 1) * HD,
                                     half:half + 1],
                            ks_col)

            kf_c = {}
            vp_c = {}

            def do_k(ci, xt):
                kfs = []
                for sub in range(NSUB):
                    ps = pps.tile([P, CG], F32, tag="pps", name="kps")
                    for kt in range(NKT):
                        nc.tensor.matmul(
                            ps[:],
                            xt(kt)[:, sub * P:(sub + 1) * P],
                            w_sb["k"][:, kt * CG:(kt + 1) * CG],
                            start=(kt == 0), stop=(kt == NKT - 1))
                    t = tmp.tile([P, CG], BF, tag="t", name="t_kb")
                    nc.vector.tensor_tensor(t[:], ps[:], bk_rep[:], Alu.add)
                    e = tmp.tile([P, CG], BF, tag="t", name="t_e")
                    nc.scalar.activation(e[:], t[:], Act.Exp)
                    m = tmp.tile([P, CG], BF, tag="t", name="t_m")
                    nc.vector.tensor_scalar(m[:], e[:], 1.0, None, Alu.min)
                    kf = kfpool.tile([P, CG], BF, tag="kf", name="kf")
                    # kf = max(t,0) + m
                    nc.vector.scalar_tensor_tensor(
                        kf[:], t[:], 0.0, m[:], Alu.max, Alu.add)
                    kfs.append(kf)
                kf_c[ci] = kfs

            def do_v(ci, xt):
                vps = []
                for sub in range(NSUB):
                    ps = pps.tile([P, CG], F32, tag="pps", name="vps")
                    for kt in range(NKT):
                        nc.tensor.matmul(
                            ps[:],
                            xt(kt)[:, sub * P:(sub + 1) * P],
                            w_sb["v"][:, kt * CG:(kt + 1) * CG],
                            start=(kt == 0), stop=(kt == NKT - 1))
                    vp = vp_tiles[(ci % 2) * NSUB + sub]
                    nc.scalar.copy(
                        vp[:].rearrange("p (h e) -> p h e", e=HE)[:, :, :HD],
                        ps[:].rearrange("p (h e) -> p h e", e=HD))
                    vps.append(vp)
                vp_c[ci] = vps

            def do_q(ci, xt):
                for ct in range(NCT):
                    ps = pps.tile([P, CHUNK], F32, tag="pps", name="qps")
                    for kt in range(NKT):
                        nc.tensor.matmul(
                            ps[:],
                            w_sb["q"][:, kt * CG + ct * P:
                                      kt * CG + (ct + 1) * P],
                            xt(kt)[:],
                            start=(kt == 0), stop=(kt == NKT - 1))
                    bcol = bq_sb[:, ct:ct + 1]
                    e = tmp.tile([P, CHUNK], BF, tag="t", name="t_qe")
                    nc.scalar.activation(e[:], ps[:], Act.Exp, bias=bcol)
                    m = tmp.tile([P, CHUNK], BF, tag="t", name="t_qm")
                    nc.vector.tensor_scalar(m[:], e[:], 1.0, None, Alu.min)
                    r = tmp.tile([P, CHUNK], BF, tag="t", name="t_qr")
                    nc.vector.tensor_scalar(
                        r[:], ps[:], bcol, 0.0, Alu.add, Alu.max)
                    q0 = (ci * NCT + ct) * CHUNK
                    nc.vector.tensor_tensor(
                        qft_all[:, q0:q0 + CHUNK], m[:], r[:], Alu.add)

            def do_kv(ci):
                # kv accumulation (head pairs: M=128, N=2*HE)
                kfs, vps = kf_c.pop(ci), vp_c.pop(ci)
                for hp in range(NH // 2):
                    kvt = sps.tile([P, 2 * HE], F32, tag="sps", name="kvt")
                    for sub in range(NSUB):
                        nc.tensor.matmul(
                            kvt[:],
                            kfs[sub][:, hp * P:(hp + 1) * P],
                            vps[sub][:, hp * 2 * HE:(hp + 1) * 2 * HE],
                            start=(sub == 0), stop=(sub == NSUB - 1))
                    # good quadrants: rows 0:64 cols 0:HE (head 2hp),
                    # rows 64:128 cols HE:2HE (head 2hp+1)
                    a0 = (2 * hp) * HE
                    nc.vector.tensor_tensor(
                        kv_sb[:, a0:a0 + HE], kv_sb[:, a0:a0 + HE],
                        kvt[0:HD, 0:HE], Alu.add)
                    a1 = (2 * hp + 1) * HE
                    nc.vector.tensor_tensor(
                        kv_sb[:, a1:a1 + HE], kv_sb[:, a1:a1 + HE],
                        kvt[HD:P, HE:2 * HE], Alu.add)

            # chunk 0/1 interleaved at projection level: k needs only wk,
            # so both chunks' k-projections run while wv/wq still stream in
            xts = {0: lambda kt: xtb0[:, kt * CHUNK:(kt + 1) * CHUNK],
                   1: lambda kt: xtb1[:, kt * CHUNK:(kt + 1) * CHUNK]}
            do_k(0, xts[0])
            do_k(1, xts[1])
            do_v(0, xts[0])
            do_v(1, xts[1])
            do_q(0, xts[0])
            do_kv(0)
            do_q(1, xts[1])
            do_kv(1)
            for ci in range(2, NCHUNK):
                xtb = load_xt(ci)
                xt = lambda kt, t=xtb: t[:, kt * CHUNK:(kt + 1) * CHUNK]
                do_k(ci, xt)
                do_v(ci, xt)
                if ci < NCHUNK - 1:
                    do_q(ci, xt)
                    do_kv(ci)
                else:
                    # last chunk: kv first so the kv-block build (DVE)
                    # overlaps the final q matmuls (PE)
                    do_kv(ci)
                    build_kvblocks()
                    do_q(ci, xt)

            # ---- tail: numerator/denominator + divide + store ----
            for cj in range(NCHUNK):
                tok0 = cj * CHUNK
                pd = sps.tile([P, NSUB * 2 * NCT], F32, tag="sps", name="pd")
                pns = []
                for sub in range(NSUB):
                    pn = pps.tile([P, CG], F32, tag="pps", name="pn")
                    for ct in range(NCT):
                        q0 = (cj * NCT + ct) * CHUNK + sub * P
                        nc.tensor.matmul(
                            pn[:, ct * P:(ct + 1) * P],
                            qft_all[:, q0:q0 + P], kvbn[ct][:],
                            start=True, stop=True)
                        nc.tensor.matmul(
                            pd[:, sub * 2 * NCT + ct * 2:
                               sub * 2 * NCT + (ct + 1) * 2],
                            qft_all[:, q0:q0 + P], kvbd[ct][:],
                            start=True, stop=True)
                    pns.append(pn)
                rc = rcpool.tile([P, NSUB * 2 * NCT], BF, tag="rc",
                                 name="rc")
                with nc.allow_low_precision(
                        reason="bf16 recip: denominators are O(1e3) sums"):
                    nc.vector.reciprocal(rc[:], pd[:])
                for sub in range(NSUB):
                    pn = pns[sub]
                    # PSUM f32 -> SBUF bf16 on ACT; mul runs all-bf16 on DVE
                    pnc = tmp.tile([P, CG], BF, tag="t", name="pnc")
                    nc.scalar.copy(pnc[:], pn[:])
                    ot = outpool.tile([P, CG], BF, tag="out", name="osb")
                    rcs = rc[:, sub * 2 * NCT:(sub + 1) * 2 * NCT]
                    rcb = rcs.unsqueeze(2).broadcast_to([P, 2 * NCT, HD])
                    pn3 = pnc[:].rearrange("p (h e) -> p h e", e=HD)
                    ot3 = ot[:].rearrange("p (h e) -> p h e", e=HD)
                    nc.vector.tensor_tensor(ot3, pn3, rcb, Alu.mult)
                    nc.sync.dma_start(
                        out_d[tok0 + sub * P:tok0 + (sub + 1) * P, :], ot[:])

    nc.compile()
    return nc


def _get_nc():
    global _CACHED_NC
    if _CACHED_NC is None:
        _CACHED_NC = _build()
    return _CACHED_NC


def _pack_w(w):
    # [D, CG] -> [P, kt-major NKT*CG]: row p, col kt*CG+c = w[kt*P+p, c]
    return np.ascontiguousarray(
        w.reshape(NKT, P, CG).transpose(1, 0, 2).reshape(P, NKT * CG))


def _make_in_maps(hidden_states, Wq, bq, Wk, bk, Wv, bv):
    hs = np.asarray(hidden_states, np.float32)
    wq = np.asarray(Wq, np.float32).astype(BF16)
    wk = np.asarray(Wk, np.float32).astype(BF16)
    wv = np.asarray(Wv, np.float32).astype(BF16)
    bqf = np.asarray(bq, np.float32)
    bkf = np.asarray(bk, np.float32).astype(BF16)
    bvf = np.asarray(bv, np.float32)
    # [S, D] -> [(chunk, p), kt-major col]:
    # row ci*P+p, col kt*CHUNK+c = X[ci*CHUNK+c, kt*P+p]
    xts = [np.ascontiguousarray(
        hs[b].astype(BF16).reshape(NCHUNK, CHUNK, NKT, P)
        .transpose(0, 3, 2, 1).reshape(NCHUNK * P, NKT * CHUNK))
        for b in range(B)]
    in_maps = []
    for c in range(NCORES):
        b, g = divmod(c, 2)
        sl = slice(g * CG, (g + 1) * CG)
        in_maps.append({
            "xt": xts[b],
            "wq": _pack_w(wq[:, sl]),
            "wk": _pack_w(wk[:, sl]),
            "wv": _pack_w(wv[:, sl]),
            "bq": np.ascontiguousarray(bqf[sl]),
            "bk": np.ascontiguousarray(bkf[sl]).reshape(1, CG),
            "bv": np.ascontiguousarray(bvf[sl]).reshape(1, CG),
        })
    return in_maps


def _run(in_maps, **kwargs):
    from concourse.bass_utils import run_bass_kernel_spmd
    nc = _get_nc()
    return run_bass_kernel_spmd(nc, in_maps, core_ids=list(range(NCORES)),
                                **kwargs)


def _assemble(results):
    out = np.empty((B, S, D), np.float32)
    for c in range(NCORES):
        b, g = divmod(c, 2)
        out[b, :, g * CG:(g + 1) * CG] = np.asarray(
            results[c]["out"], dtype=np.float32)
    return out


def kernel(hidden_states, Wq, bq, Wk, bk, Wv, bv):
    in_maps = _make_in_maps(hidden_states, Wq, bq, Wk, bk, Wv, bv)
    res = _run(in_maps)
    return _assemble(res.results)

